# revision 1
# baseline (speedup 1.0000x reference)
"""Trainium2 Bass kernel for nn_AttentionLayer (B=8,T=12,S=512,D=128,H=8).

Data-parallel over batch: core b handles (query/key/value)[b], T=12 steps.

Design (v2 — ACT-exp-bound pipeline):
  - All matmul operands bf16 (fp32 PSUM accumulate); host pre-transposes
    inputs to [T, D, S] bf16.
  - Scores per head-pair band g at K=32 via PE row tiling: waves of 2
    concurrent row-tiled matmuls (tile_position=(32g,0) / (32g+32,0)) into
    a [P, 2, 512] fp32 psum pair (one bank per row tile). 16 waves per
    timestep: (parity p, gpair gp, ktile st). Ping-pong via bufs=2.
  - Exp on ScalarE: one ACTIVATE per wave (FD=1024, scale=0.25) psum ->
    exp_sb bf16. ACT does nothing else; it is the roofline engine
    (~18.3us/t).
  - attnV: per head 4 accumulating matmuls (K=128, M=32) col-tiled to
    offset 32g of the parity's av bank (tile_position=(0,32g)).
    v_sb stationary is [P, 32]: 16 v cols, a ones col (sumexp row), and
    15 zero cols so the whole 32-row block is written (no stale psum in
    the gaps). attnV matmuls are drained 2 per wave behind the exps.
  - Softmax denominators: sumexp rows gathered by a one-hot matmul,
    reciprocal on DVE (reciprocal_approx_fast), broadcast back over each
    head's 32 rows by a one-hot matmul, one tensor_mul per parity.
  - Out-proj: 2 accumulating matmuls (WoE/WoO with zeroed non-head rows)
    + cvec (= bv@Wo + bo) fused into the psum eviction.
  - Norm chain + outproj of step t run during t+1's waves.
"""

import sys

sys.path.insert(0, "/opt/trn_rl_repo")

import numpy as np
import ml_dtypes

B, T, S, D = 8, 12, 512, 128
H, HD = 8, 16
NT = S // 128  # 4 k-tiles of 128
P = 128
WARMUP_MM = 16  # HAM warm-up burst at kernel start (N=512 each, ~6us cold)

# custom-DVE exp: scores are produced pre-scaled by C0_SCALE (folded into
# Wk on the host) and pre-biased by -64 (hijacked parity row), so psum
# holds y = score*32*log2(e) - 64. ACT un-does this via scale/bias; the
# DVE op consumes y directly (magic-add floor + quad 2^f + bf16 packing).
EXP_NAME = "EXPB16ATT"
C0_SCALE = float(32.0 * np.log2(np.e))
GAM = 0.002690
ALPHA = 64.0 - 4096.0 * GAM + 16256.0
ACT_SCALE = float(0.25 / C0_SCALE)
ACT_BIAS = float(64.0 * 0.25 / C0_SCALE)


def register_exp_op():
    import concourse.dve_ops as dom
    from concourse.dve_spec import Spec, Src0, C0, C1, C2, One, lower
    from concourse.dve_uop import DveOpSpec

    for op in dom.OPS:
        if op.name == EXP_NAME:
            return op

    t = Src0 + C0
    p = t - C0
    f = Src0 - p
    w = (C2 * f + One) * f + (p + C1)

    def _ref(in0, in1, c0, c1, c2):
        yv = np.asarray(in0, np.float32)
        cm = np.float32(1.5 * 2**30)
        tv = (yv + cm).astype(np.float32)
        pv = (tv - cm).astype(np.float32)
        fv = (yv - pv).astype(np.float32)
        return (
            (np.float32(c2) * fv + np.float32(1.0)).astype(np.float32) * fv
            + (pv + np.float32(c1)).astype(np.float32)
        ).astype(np.float32)

    spec = Spec(body=w, reference=_ref)
    shas = {}
    for ver in ("v3", "v4"):
        try:
            uops = lower(spec, ver=ver)
            shas[ver] = DveOpSpec(
                name=EXP_NAME,
                opcode=max(dom._SUB_OPCODE_FOR_NAME.values()) + 1,
                uops=uops,
                rd1_en=False,
            ).sha(ver)
        except Exception:
            pass
    op = dom.DveOp(EXP_NAME, spec, subdim=False, uops_sha=shas)
    dom.OPS.append(op)
    dom._SUB_OPCODE_FOR_NAME[EXP_NAME] = (
        max(dom._SUB_OPCODE_FOR_NAME.values()) + 1
    )
    dom.CUSTOM_DVE_SPECS[EXP_NAME] = spec
    return op


def build_bass():
    from contextlib import ExitStack

    import concourse.bass as bass
    from concourse import bacc, mybir
    import concourse.tile as tile

    fp32 = mybir.dt.float32
    bf16 = mybir.dt.bfloat16
    i16 = mybir.dt.int16
    AF = mybir.ActivationFunctionType
    expop = register_exp_op()

    nc = bacc.Bacc(None, target_bir_lowering=False)

    qT_d = nc.declare_dram_parameter("qT", [T, D, S], bf16, isOutput=False)
    kT_d = nc.declare_dram_parameter("kT", [T, D, S], bf16, isOutput=False)
    vT_d = nc.declare_dram_parameter("vT", [T, D, S], bf16, isOutput=False)
    # packed constants: W5 = [Wq, Wk, Wv, WoE, WoO]; sel = [selE, selO];
    # sel8 = [sel8E, sel8O]; vec = [bqe, bqo, bk, cvec, maskE, maskO]
    W5_d = nc.declare_dram_parameter("W5", [D, 5, D], bf16, isOutput=False)
    sel_d = nc.declare_dram_parameter("sel", [D, 2, 8], bf16, isOutput=False)
    sel8_d = nc.declare_dram_parameter("sel8", [8, 2, D], bf16, isOutput=False)
    vec_d = nc.declare_dram_parameter("vec", [D, 11], fp32, isOutput=False)
    # output is produced transposed ([d, s] per t); host untransposes
    out_d = nc.declare_dram_parameter("out", [T, D, S], fp32, isOutput=True)

    with ExitStack() as ctx:
        tc = ctx.enter_context(tile.TileContext(nc))
        consts = ctx.enter_context(tc.tile_pool(name="consts", bufs=1))
        io = ctx.enter_context(tc.tile_pool(name="io", bufs=3))
        proj = ctx.enter_context(tc.tile_pool(name="proj", bufs=2))
        expp = ctx.enter_context(tc.tile_pool(name="expp", bufs=2))
        avsbp = ctx.enter_context(tc.tile_pool(name="avsbp", bufs=2))
        normp = ctx.enter_context(tc.tile_pool(name="normp", bufs=2))
        outp = ctx.enter_context(tc.tile_pool(name="outp", bufs=2))
        pssc = ctx.enter_context(
            tc.tile_pool(name="pssc", bufs=2, space=bass.MemorySpace.PSUM)
        )
        psav = ctx.enter_context(
            tc.tile_pool(name="psav", bufs=2, space=bass.MemorySpace.PSUM)
        )
        pspj = ctx.enter_context(
            tc.tile_pool(name="pspj", bufs=2, space=bass.MemorySpace.PSUM)
        )

        # ---- HAM warm-up: dense bf16 matmul burst on a zeroed tile ----
        # (no DMA dependency, so it overlaps the constant/input loads)
        warm_sb = consts.tile([P, S], bf16)
        nc.vector.memset(warm_sb, 0.0)
        for i in range(WARMUP_MM):
            ps_w = pspj.tile([P, S], fp32, tag="pj", name=f"warm{i}")
            nc.tensor.matmul(ps_w, warm_sb[:, 0:D], warm_sb,
                             start=True, stop=True)

        # ---- constants (loaded once, batched) ----
        W5_sb = consts.tile([D, 5, D], bf16)
        sel_sb = consts.tile([D, 2, 8], bf16)
        sel8_sb = consts.tile([8, 2, D], bf16)
        vec_sb = consts.tile([D, 11], fp32)
        nc.sync.dma_start(out=W5_sb, in_=W5_d[:])
        nc.sync.dma_start(out=sel_sb, in_=sel_d[:])
        nc.sync.dma_start(out=sel8_sb, in_=sel8_d[:])
        nc.sync.dma_start(out=vec_sb, in_=vec_d[:])
        Wq_sb = W5_sb[:, 0, :]
        Wk_sb = W5_sb[:, 1, :]
        Wv_sb = W5_sb[:, 2, :]
        WoE_sb = W5_sb[:, 3, :]
        WoO_sb = W5_sb[:, 4, :]
        selE_sb = sel_sb[:, 0, :]
        selO_sb = sel_sb[:, 1, :]
        sel8E_sb = sel8_sb[:, 0, :]
        sel8O_sb = sel8_sb[:, 1, :]
        bqe_sb = vec_sb[:, 0:1]
        bqo_sb = vec_sb[:, 1:2]
        c_sb = vec_sb[:, 3:4]
        maskE_sb = vec_sb[:, 4:5]
        maskO_sb = vec_sb[:, 5:6]
        bkE_sb = vec_sb[:, 6:7]
        bkO_sb = vec_sb[:, 7:8]
        maskKE_sb = vec_sb[:, 8:9]
        maskKO_sb = vec_sb[:, 9:10]
        actb_sb = vec_sb[:, 10:11]

        mlt, add = mybir.AluOpType.mult, mybir.AluOpType.add

        # state dicts for software pipelining
        av_tiles = {}       # (rt, p) -> psum av tile
        av_sb_tiles = {}    # (rt, p) -> sbuf evicted av
        exp_tiles = {}      # rt -> exp_sb tile
        vsb_tiles = {}      # rt -> v_sb tile
        norm_steps = {}     # rt -> list of closures
        pending_av = []     # deque of (rt, p, g, st) matmuls not yet emitted
        av_count = {}       # (rt, p) -> MMs emitted so far

        def emit_av_mm():
            if not pending_av:
                return
            rt, p, g, st = pending_av.pop(0)
            av_ps = av_tiles[(rt, p)]
            v_sb = vsb_tiles[rt]
            exp_sb = exp_tiles[rt]
            h = 2 * g + p
            nc.tensor.matmul(
                av_ps[32 * g : 32 * g + 32, :],
                v_sb[:, st, h, :],
                exp_sb[:, p, g, st, :],
                start=(st == 0),
                stop=(st == NT - 1),
                tile_position=(0, 32 * g),
            )
            n = av_count.get((rt, p), 0) + 1
            av_count[(rt, p)] = n
            if n == 16:
                # parity complete: evict to SBUF (bf16)
                av_sb = avsbp.tile([P, S], bf16, tag=f"av{p}",
                                   name=f"avsb{rt}_{p}")
                nc.vector.tensor_copy(av_sb, av_ps)
                av_sb_tiles[(rt, p)] = av_sb
                if p == 1:
                    norm_steps[rt] = make_norm_steps(rt)

        def make_norm_steps(rt):
            avE = av_sb_tiles.pop((rt, 0))
            avO = av_sb_tiles.pop((rt, 1))
            state = {}

            def s0():  # gather the 8 sumexp rows
                ps_g = pspj.tile([P, S], fp32, tag="pj", name=f"ps_g{rt}")
                nc.tensor.matmul(ps_g[0:8, :], selE_sb, avE,
                                 start=True, stop=False)
                nc.tensor.matmul(ps_g[0:8, :], selO_sb, avO,
                                 start=False, stop=True)
                state["ps_g"] = ps_g

            def s1():  # reciprocal on DVE
                recip_f = normp.tile([8, S], fp32, tag="recf",
                                     name=f"recf{rt}")
                nc.vector.reciprocal_approx_fast(
                    out=recip_f, in_=state["ps_g"][0:8, :]
                )
                recip_b = normp.tile([8, S], bf16, tag="recb",
                                     name=f"recb{rt}")
                nc.vector.tensor_copy(recip_b, recip_f)
                state["recip"] = recip_b

            def s2():  # broadcast reciprocals over each head's 32 rows
                ps_RE = pspj.tile([P, S], fp32, tag="pj", name=f"ps_RE{rt}")
                nc.tensor.matmul(ps_RE, sel8E_sb, state["recip"],
                                 start=True, stop=True)
                ps_RO = pspj.tile([P, S], fp32, tag="pj", name=f"ps_RO{rt}")
                nc.tensor.matmul(ps_RO, sel8O_sb, state["recip"],
                                 start=True, stop=True)
                state["ps_RE"], state["ps_RO"] = ps_RE, ps_RO

            def s3():  # normalize
                aE = normp.tile([P, S], bf16, tag="aE", name=f"aE{rt}")
                aO = normp.tile([P, S], bf16, tag="aO", name=f"aO{rt}")
                nc.vector.tensor_mul(aE, avE, state["ps_RE"])
                nc.vector.tensor_mul(aO, avO, state["ps_RO"])
                state["aE"], state["aO"] = aE, aO

            def s4():  # out-projection + bias + store
                ps_ot = pspj.tile([P, S], fp32, tag="pj", name=f"ps_ot{rt}")
                nc.tensor.matmul(ps_ot, WoE_sb, state["aE"],
                                 start=True, stop=False)
                nc.tensor.matmul(ps_ot, WoO_sb, state["aO"],
                                 start=False, stop=True)
                o_sb = outp.tile([P, S], fp32, tag="o", name=f"o{rt}")
                nc.vector.tensor_scalar_add(o_sb, ps_ot, c_sb)
                nc.sync.dma_start(out=out_d[rt % T], in_=o_sb)

            return [s0, s1, s2, s3, s4]

        io_tiles = {}
        proj_tiles = {}

        def emit_loads(rt):
            qT_in = io.tile([D, S], bf16, tag="qT_in")
            kT_in = io.tile([D, S], bf16, tag="kT_in")
            vT_in = io.tile([D, S], bf16, tag="vT_in")
            nc.sync.dma_start(out=qT_in, in_=qT_d[rt % T])
            nc.sync.dma_start(out=kT_in, in_=kT_d[rt % T])
            nc.sync.dma_start(out=vT_in, in_=vT_d[rt % T])
            io_tiles[rt] = (qT_in, kT_in, vT_in)

        def emit_proj_q(rt):
            qT_in = io_tiles[rt][0]
            ps_q = pspj.tile([P, S], fp32, tag="pj", name=f"ps_q{rt}")
            nc.tensor.matmul(ps_q, Wq_sb, qT_in, start=True, stop=True)
            qT_ev = proj.tile([P, S], bf16, tag="qT_ev")
            qT_od = proj.tile([P, S], bf16, tag="qT_od")
            nc.vector.tensor_scalar(qT_ev, ps_q, maskE_sb, bqe_sb, mlt, add)
            nc.vector.tensor_scalar(qT_od, ps_q, maskO_sb, bqo_sb, mlt, add)
            proj_tiles.setdefault(rt, {})["q"] = (qT_ev, qT_od)

        def emit_proj_k(rt):
            # two parity variants: the other parity's leading row in each
            # 32-band is replaced by -64 (the exp-bias hijack row)
            kT_in = io_tiles[rt][1]
            ps_k = pspj.tile([P, S], fp32, tag="pj", name=f"ps_k{rt}")
            nc.tensor.matmul(ps_k, Wk_sb, kT_in, start=True, stop=True)
            kT_ev = proj.tile([P, S], bf16, tag="kT_ev")
            kT_od = proj.tile([P, S], bf16, tag="kT_od")
            nc.vector.tensor_scalar(kT_ev, ps_k, maskKE_sb, bkE_sb, mlt, add)
            nc.vector.tensor_scalar(kT_od, ps_k, maskKO_sb, bkO_sb, mlt, add)
            proj_tiles.setdefault(rt, {})["k"] = (kT_ev, kT_od)

        def emit_proj_v(rt):
            # v in natural [s, d] layout; stationary blocks [P, 32]:
            # cols 0:16 = v, col 16 = ones (sumexp), cols 17:32 = zeros
            vT_in = io_tiles[rt][2]
            ps_v = pspj.tile([P, S], fp32, tag="pj", name=f"ps_v{rt}")
            for st in range(NT):
                nc.tensor.matmul(
                    ps_v[:, st * 128 : (st + 1) * 128],
                    vT_in[:, st * 128 : (st + 1) * 128],
                    Wv_sb,
                    start=True,
                    stop=True,
                )
            v_sb = proj.tile([P, NT, H, 32], bf16, tag="v_sb")
            nc.vector.memset(v_sb, 0.0)
            nc.vector.memset(v_sb[:, :, :, HD : HD + 1], 1.0)
            nc.vector.tensor_copy(
                v_sb[:, :, :, 0:HD],
                ps_v.rearrange("p (st h j) -> p st h j", st=NT, h=H),
            )
            vsb_tiles[rt] = v_sb

        for rt in range(T):
            if rt == 0:
                emit_loads(0)
                emit_proj_q(0)
                emit_proj_k(0)
                emit_proj_v(0)
            qT_ev, qT_od = proj_tiles[rt]["q"]
            kT_ev, kT_od = proj_tiles[rt]["k"]

            exp_sb = expp.tile([P, 2, 4, NT, S], bf16, tag="exp",
                               name=f"exp{rt}")
            exp_tiles[rt] = exp_sb

            # ---- 16 waves: (parity, gpair, ktile) ----
            for p in range(2):
                qT_par = qT_ev if p == 0 else qT_od
                kT_par = kT_ev if p == 0 else kT_od
                for gp in range(2):
                    for st in range(NT):
                        w = 8 * p + 4 * gp + st
                        ps_sc = pssc.tile([P, 2, S], fp32, tag="sc")
                        for i in range(2):
                            g = 2 * gp + i
                            nc.tensor.matmul(
                                ps_sc[:, i, :],
                                kT_par[32 * g : 32 * g + 32,
                                       st * 128 : (st + 1) * 128],
                                qT_par[32 * g : 32 * g + 32, :],
                                start=True,
                                stop=True,
                                tile_position=(32 * g, 0),
                            )
                        if w % 2 == 1:
                            # split: ACT does bank 0, DVE-exp does bank 1
                            nc.scalar.activation(
                                exp_sb[:, p, 2 * gp, st, :],
                                ps_sc[:, 0, :],
                                AF.Exp,
                                scale=ACT_SCALE,
                                bias=actb_sb,
                            )
                            nc.vector._custom_dve(
                                expop,
                                out=exp_sb.bitcast(i16)[:, p, 2 * gp + 1,
                                                        st, :],
                                in0=ps_sc[:, 1, :],
                                s0=float(1.5 * 2**30),
                                s1=float(ALPHA),
                                imm2=float(GAM),
                            )
                        else:
                            nc.scalar.activation(
                                exp_sb[:, p, 2 * gp : 2 * gp + 2, st, :],
                                ps_sc,
                                AF.Exp,
                                scale=ACT_SCALE,
                                bias=actb_sb,
                            )
                        # drain 2 pending attnV matmuls behind the exps
                        emit_av_mm()
                        emit_av_mm()
                        if rt == 0 and w < 4:
                            # pipeline not primed yet: keep the PE busy so
                            # the HAM clock gate stays at 8/8
                            ps_f = pspj.tile([P, S], fp32, tag="pj",
                                             name=f"fill{w}")
                            nc.tensor.matmul(ps_f, warm_sb[:, 0:D], warm_sb,
                                             start=True, stop=True)
                        # norm chain of t-1 rides waves 4..8
                        if rt > 0 and 4 <= w <= 8:
                            norm_steps[rt - 1][w - 4]()
                        # next step's loads + projections ride waves 10..13
                        if rt + 1 < T:
                            if w == 10:
                                emit_loads(rt + 1)
                            elif w == 11:
                                emit_proj_q(rt + 1)
                            elif w == 12:
                                emit_proj_k(rt + 1)
                            elif w == 13:
                                emit_proj_v(rt + 1)
                    # gpair complete: queue its attnV groups, head-pair
                    # interleaved so drained pairs run on distinct col tiles
                    pp = p
                    if (rt, pp) not in av_tiles:
                        av_tiles[(rt, pp)] = psav.tile(
                            [P, S], fp32, tag="av", name=f"av{rt}_{pp}"
                        )
                    for st in range(NT):
                        for i in range(2):
                            pending_av.append((rt, pp, 2 * gp + i, st))

        # ---- epilogue: drain remaining attnV, last norm chain ----
        while pending_av:
            emit_av_mm()
        for step in norm_steps[T - 1]:
            step()

    nc.compile()
    return nc


def make_in_maps(query, key, value, Wq, bq, Wk, bk, Wv, bv, Wo, bo):
    f = np.float32
    bf = ml_dtypes.bfloat16
    Wo = np.asarray(Wo, f)
    c = (np.asarray(bv, f) @ Wo + np.asarray(bo, f)).reshape(D, 1)
    WoE = np.zeros((D, D), f)
    WoO = np.zeros((D, D), f)
    selE = np.zeros((D, 8), f)
    selO = np.zeros((D, 8), f)
    sel8E = np.zeros((8, D), f)
    sel8O = np.zeros((8, D), f)
    for g in range(4):
        hE, hO = 2 * g, 2 * g + 1
        WoE[32 * g : 32 * g + 16, :] = Wo[HD * hE : HD * (hE + 1), :]
        WoO[32 * g : 32 * g + 16, :] = Wo[HD * hO : HD * (hO + 1), :]
        selE[32 * g + 16, hE] = 1.0
        selO[32 * g + 16, hO] = 1.0
        sel8E[hE, 32 * g : 32 * g + 32] = 1.0
        sel8O[hO, 32 * g : 32 * g + 32] = 1.0
    maskE = np.zeros((D, 1), f)
    maskO = np.zeros((D, 1), f)
    for p in range(D):
        if (p % 32) < 16:
            maskE[p] = 1.0
        else:
            maskO[p] = 1.0
    bq = np.ascontiguousarray(bq, f).reshape(D, 1)
    Wk_s = np.asarray(Wk, f) * np.float32(C0_SCALE)
    bk_s = np.ascontiguousarray(bk, f).reshape(D, 1) * np.float32(C0_SCALE)
    W5 = np.stack(
        [np.asarray(Wq, f), Wk_s, np.asarray(Wv, f), WoE, WoO],
        axis=1,
    )
    sel = np.stack([selE, selO], axis=1)
    sel8 = np.stack([sel8E, sel8O], axis=1)
    # exp-bias hijack rows: q side gets 1.0 at the masked leading row of
    # the opposite parity; k side gets -64.0 there (maskK zeroes it first)
    bqe_v = bq * maskE
    bqo_v = bq * maskO
    maskKE = np.ones((D, 1), f)
    maskKO = np.ones((D, 1), f)
    bkE_v = bk_s.copy()
    bkO_v = bk_s.copy()
    for g in range(4):
        bqe_v[32 * g + 16] = 1.0
        bqo_v[32 * g] = 1.0
        maskKE[32 * g + 16] = 0.0
        bkE_v[32 * g + 16] = -64.0
        maskKO[32 * g] = 0.0
        bkO_v[32 * g] = -64.0
    actb = np.full((D, 1), np.float32(ACT_BIAS), f)
    vec = np.concatenate(
        [bqe_v, bqo_v, bk_s, c, maskE, maskO, bkE_v, bkO_v, maskKE, maskKO,
         actb],
        axis=1,
    )
    shared = {
        "W5": np.ascontiguousarray(W5).astype(bf),
        "sel": np.ascontiguousarray(sel).astype(bf),
        "sel8": np.ascontiguousarray(sel8).astype(bf),
        "vec": np.ascontiguousarray(vec),
    }
    in_maps = []
    for b in range(B):
        m = dict(shared)
        m["qT"] = np.ascontiguousarray(
            np.asarray(query[b], f).transpose(0, 2, 1)
        ).astype(bf)
        m["kT"] = np.ascontiguousarray(
            np.asarray(key[b], f).transpose(0, 2, 1)
        ).astype(bf)
        m["vT"] = np.ascontiguousarray(
            np.asarray(value[b], f).transpose(0, 2, 1)
        ).astype(bf)
        in_maps.append(m)
    return in_maps


def kernel(query, key, value, Wq, bq, Wk, bk, Wv, bv, Wo, bo):
    from concourse.bass_utils import run_bass_kernel_spmd

    nc = build_bass()
    in_maps = make_in_maps(query, key, value, Wq, bq, Wk, bk, Wv, bv, Wo, bo)
    res = run_bass_kernel_spmd(nc, in_maps, core_ids=list(range(B)))
    out = np.stack(
        [res.results[i]["out"].transpose(0, 2, 1) for i in range(B)]
    )
    return out

